# revision 9
# baseline (speedup 1.0000x reference)
"""CMPGNN message-passing kernel for 8 Trainium2 NeuronCores.

v2: batched SWDGE gathers (dma_gather) instead of per-tile indirect DMA.

Layout (host):
- core deal: global degree-desc rank r -> core r%8.
- per-node "lane" in 0..3 chosen by a host-side refinement balancing
  per-(target block, lane) in-edge counts; repaired to exactly 3125 real
  nodes per (core, lane) so pad rows sit at identical srows on every
  core (pure SPMD, one NEFF).
- per-core table order: 4 contiguous lane regions of HS=3136 rows
  (= AllGather chunks); within a lane, degree desc (even lanes) / asc
  (odd lanes) so 128-row blocks stay degree-homogeneous; 11 pad rows at
  each lane tail.  Global table pid = lane*8*HS + core*HS + local, so a
  lane is a contiguous quarter of the gathered table and its lane-local
  index fits int16 (dma_gather requirement).

Device (per layer):
- table build (bf16 matmul per block) writes [h3 | -(h3+h4)] rows;
  AllGather fires per chunk as its rows land (chunk == gather lane, so
  lane-k gathers only wait on chunk k).
- edge pass over groups of consecutive blocks: one dma_gather per
  (group, lane) into a lane-major rectangular group buffer (band height
  shared within the group), then per-lane DVE ops (dot, reduce, gate
  with a stride-0 4D broadcast of h4), per-group sigmoid, and per-block
  PSUM identity-matmul scatter + relu(Q@Wf.T) + L2 normalize.  Layer 2
  also emits logits inside the edge pass; log_softmax runs batched.
"""
import numpy as np

import concourse.bass as bass
import concourse.bacc as bacc
import concourse.tile as tile
from concourse import mybir
from concourse.bass_utils import run_bass_kernel_spmd

N, E, F_IN, H, C = 100000, 1250000, 512, 64, 40
NCORE, P, S, NL = 8, 128, 98, 4
HS, SHARD = 3136, 12544
NTAB = NCORE * SHARD
NREAL = 3125
GB_MAX = 10
W_MAX = 128
KCH = F_IN // P
F32 = mybir.dt.float32
BF16 = mybir.dt.bfloat16
I16 = mybir.dt.int16
AX = mybir.AxisListType
ALU = mybir.AluOpType
ACTF = mybir.ActivationFunctionType

# AllGather chunk k fires after the table-write pair ending at this block
AG_FIRE = {25: 0, 49: 1, 73: 2, 97: 3}
# pad rows (11 per lane tail): (block, p0, p1) to zero in QT after phase 1
PAD_ZERO = [(24, 53, 64), (48, 117, 128), (73, 53, 64), (97, 117, 128)]


# ---------------------------------------------------------------------------
# host-side prep
# ---------------------------------------------------------------------------

def _refine_lanes(row, col, deg, core_of, rounds=5, seed=0):
    rng = np.random.default_rng(seed)
    order = np.argsort(-deg, kind="stable")
    rank_in_core = np.empty(N, np.int64)
    for c in range(NCORE):
        m = order[core_of[order] == c]
        rank_in_core[m] = np.arange(len(m))
    lane = (rank_in_core % NL).astype(np.int8)

    eo = np.argsort(row, kind="stable")
    col_s = col[eo]
    starts = np.searchsorted(row[eo], np.arange(N + 1))

    blk = np.empty(N, np.int64)
    for c in range(NCORE):
        nodes_c = np.nonzero(core_of == c)[0]
        for k in range(NL):
            nk = nodes_c[lane[nodes_c] == k]
            o = np.argsort(-deg[nk] if k % 2 == 0 else deg[nk], kind="stable")
            blk[nk[o]] = c * S + (k * HS + np.arange(len(nk))) // P
    dmax = np.zeros(NCORE * S, np.int64)
    np.maximum.at(dmax, blk, deg)
    t = (dmax[:, None] // NL
         + (np.arange(NL)[None, :] < (dmax[:, None] % NL))).astype(np.int32)
    tv = t[blk]
    cnt = np.stack([np.bincount(col[lane[row] == k], minlength=N)
                    for k in range(NL)], 1).astype(np.int32)
    lane_sizes = np.zeros((NCORE, NL), np.int64)
    np.add.at(lane_sizes, (core_of, lane), 1)

    for rnd in range(rounds):
        moved = 0
        for r in rng.permutation(N):
            a, b = starts[r], starts[r + 1]
            if a == b:
                continue
            tn = col_s[a:b]
            cq = cnt[tn]
            qq = tv[tn]
            kr = lane[r]
            rm = (np.maximum(cq[:, kr] - 1 - qq[:, kr], 0) ** 2
                  - np.maximum(cq[:, kr] - qq[:, kr], 0) ** 2).sum()
            best_k, best_delta = kr, 0
            for k in range(NL):
                if k == kr or lane_sizes[core_of[r], k] >= HS - 1:
                    continue
                ad = (np.maximum(cq[:, k] + 1 - qq[:, k], 0) ** 2
                      - np.maximum(cq[:, k] - qq[:, k], 0) ** 2).sum()
                if rm + ad < best_delta:
                    best_delta, best_k = rm + ad, k
            if best_k != kr:
                cnt[tn, kr] -= 1
                cnt[tn, best_k] += 1
                lane_sizes[core_of[r], kr] -= 1
                lane_sizes[core_of[r], best_k] += 1
                lane[r] = best_k
                moved += 1
        if moved < N // 300:
            break

    # repair to exactly NREAL per (core, lane)
    for c in range(NCORE):
        nodes_c = np.nonzero(core_of == c)[0]
        for _ in range(8 * NL):
            sizes = np.bincount(lane[nodes_c], minlength=NL)
            hi = int(np.argmax(sizes))
            lo = int(np.argmin(sizes))
            if sizes[hi] <= NREAL:
                break
            cand = nodes_c[lane[nodes_c] == hi]
            nmove = min(int(sizes[hi] - NREAL), int(NREAL - sizes[lo]))
            costs = np.zeros(len(cand), np.int64)
            for i, r in enumerate(cand):
                a, b = starts[r], starts[r + 1]
                tn = col_s[a:b]
                costs[i] = (np.maximum(cnt[tn, lo] + 1 - tv[tn, lo], 0) ** 2).sum()
            mv = cand[np.argsort(costs, kind="stable")[:nmove]]
            for r in mv:
                a, b = starts[r], starts[r + 1]
                tn = col_s[a:b]
                cnt[tn, hi] -= 1
                cnt[tn, lo] += 1
            lane[mv] = lo
        assert (np.bincount(lane[nodes_c], minlength=NL) == NREAL).all()
    return lane


def _prep(x, edge_index):
    x = np.asarray(x, np.float32)
    row = np.asarray(edge_index[0], dtype=np.int64)
    col = np.asarray(edge_index[1], dtype=np.int64)
    deg = np.bincount(col, minlength=N)

    order = np.argsort(-deg, kind="stable")
    core_of = np.empty(N, np.int64)
    core_of[order] = np.arange(N) % NCORE

    lane = _refine_lanes(row, col, deg, core_of).astype(np.int64)

    srow_of = np.empty(N, np.int64)
    for c in range(NCORE):
        nodes_c = np.nonzero(core_of == c)[0]
        for k in range(NL):
            nk = nodes_c[lane[nodes_c] == k]
            o = np.argsort(-deg[nk] if k % 2 == 0 else deg[nk], kind="stable")
            nk = nk[o]
            assert len(nk) == NREAL
            srow_of[nk] = k * HS + np.arange(NREAL)
    s_of = srow_of // P
    p_of = srow_of % P
    q_of = core_of * HS + (srow_of % HS)

    cnt = np.stack([np.bincount(col[lane[row] == k], minlength=N)
                    for k in range(NL)], 1).astype(np.int64)
    That = np.zeros((S, NL), np.int64)
    np.maximum.at(That, (np.repeat(s_of, NL), np.tile(np.arange(NL), N)),
                  cnt.reshape(-1))
    That = np.maximum(That, 1)

    groups = []
    s = 0
    while s < S:
        nb = 1
        cur = That[s].copy()
        while s + nb < S and nb < GB_MAX:
            cand = np.maximum(cur, That[s + nb])
            w = int(cand.sum()) * (nb + 1)
            waste = w - int(That[s:s + nb + 1].sum())
            if w > W_MAX or waste > 2 * (nb + 1):
                break
            cur = cand
            nb += 1
        groups.append([s, nb, cur.copy()])
        s += nb

    tvec_of_s = np.zeros((S, NL), np.int64)
    group_meta = []
    col16 = 0
    for s0, nb, tvec in groups:
        tvec_of_s[s0:s0 + nb] = tvec
        tvec = [int(t) for t in tvec]
        coffs = []
        off = 0
        for k in range(NL):
            coffs.append(off)
            off += P * tvec[k] * nb // 16
        group_meta.append(dict(s0=s0, nb=nb, tvec=tvec, coffs=coffs,
                               col_off=col16, col_cnt=off,
                               W_band=sum(tvec) * nb))
        col16 += off
    idx16_cols = col16
    W_buf = max(g["W_band"] for g in group_meta)
    R_max = max(g["nb"] * t for g in group_meta for t in g["tvec"])
    X_max = max(g["col_cnt"] for g in group_meta)

    elane = lane[row]
    key = col * NL + elane
    eo2 = np.argsort(key, kind="stable")
    kstart = np.searchsorted(key[eo2], np.arange(N * NL + 1))
    e_rank = np.empty(E, np.int64)
    e_rank[eo2] = np.arange(E) - kstart[key[eo2]]
    e_s = s_of[col]
    e_p = p_of[col]
    e_core = core_of[col]
    assert (e_rank < tvec_of_s[e_s, elane]).all()

    node_at = np.full((NCORE, SHARD), -1, np.int64)
    node_at[core_of, srow_of] = np.arange(N)

    idx16_hosts, xT_hosts = [], []
    for c in range(NCORE):
        q_pad = c * HS + NREAL
        idx16 = np.zeros((16, max(idx16_cols, 1)), np.int16)
        mc = e_core == c
        for g in group_meta:
            s0, nb, tvec = g["s0"], g["nb"], g["tvec"]
            sel_g = mc & (e_s >= s0) & (e_s < s0 + nb)
            for k in range(NL):
                tk = tvec[k]
                if tk == 0:
                    continue
                vals = np.full((nb, tk, P), q_pad, np.int32)
                m = sel_g & (elane == k)
                vals[e_s[m] - s0, e_rank[m], e_p[m]] = q_of[row[m]]
                n = nb * tk * P
                wrapped = vals.reshape(n // 16, 16).T.astype(np.int16)
                co = g["col_off"] + g["coffs"][k]
                idx16[:, co: co + n // 16] = wrapped
        # replicate across the 8 GPSIMD cores' 16-partition windows
        idx16_hosts.append(np.tile(idx16, (NCORE, 1)))
        ids = node_at[c]
        xs = np.zeros((SHARD, F_IN), np.float32)
        mm = ids >= 0
        xs[mm] = x[ids[mm]]
        xT_hosts.append(np.ascontiguousarray(xs.T))

    plan = dict(groups=group_meta, idx16_cols=idx16_cols, W_buf=int(W_buf),
                R_max=int(R_max), X_max=int(X_max),
                tiles=int(sum(g["W_band"] for g in group_meta)))
    return plan, idx16_hosts, xT_hosts, node_at


# ---------------------------------------------------------------------------
# device kernel
# ---------------------------------------------------------------------------

def _build(plan, repeat=1):
    nc = bacc.Bacc("TRN2", target_bir_lowering=False, num_swdge_queues=4,
                   dynamic_dma_scratch_size=32768)
    groups = plan["groups"]
    X16 = max(plan["idx16_cols"], 1)
    W_buf, R_max, X_max = plan["W_buf"], plan["R_max"], plan["X_max"]

    xT_e = nc.declare_dram_parameter("xT", [F_IN, SHARD], BF16, isOutput=False)
    idx16_e = nc.declare_dram_parameter("idx16", [P, X16], I16, isOutput=False)
    winT_e = nc.declare_dram_parameter("winT", [F_IN, H], BF16, isOutput=False)
    binc_e = nc.declare_dram_parameter("b_in_col", [H, 1], F32, isOutput=False)
    bout_e = nc.declare_dram_parameter("b_out", [1, C], F32, isOutput=False)
    woutT_e = nc.declare_dram_parameter("woutT", [H, C], BF16, isOutput=False)
    wp_e = [nc.declare_dram_parameter(f"wp{l}", [H, 3 * H], BF16, isOutput=False)
            for l in range(2)]
    wfT_e = [nc.declare_dram_parameter(f"wfT{l}", [H, H], BF16, isOutput=False)
             for l in range(2)]
    out_e = nc.declare_dram_parameter("out", [SHARD, C], F32, isOutput=True)

    tabA_own = [nc.dram_tensor(f"tabA_own{l}", [SHARD, 2 * H], BF16) for l in range(2)]
    tabA_full = [nc.dram_tensor(f"tabA_full{l}", [NTAB, 2 * H], BF16,
                                addr_space="Shared") for l in range(2)]

    with tile.TileContext(nc) as tc:
        with (
            tc.tile_pool(name="const", bufs=1) as cp,
            tc.tile_pool(name="big", bufs=1) as bp,
            tc.tile_pool(name="stats", bufs=4) as stp,
            tc.tile_pool(name="xt", bufs=3) as xp,
            tc.tile_pool(name="tb", bufs=4) as tbp,
            tc.tile_pool(name="gat", bufs=2) as gp,
            tc.tile_pool(name="prod", bufs=3) as pp,
            tc.tile_pool(name="idx", bufs=3) as ip,
            tc.tile_pool(name="dsg", bufs=3) as dp,
            tc.tile_pool(name="small", bufs=6) as sp,
            tc.tile_pool(name="ps", bufs=4, space="PSUM") as ps,
            tc.tile_pool(name="psagg", bufs=2, space="PSUM") as psa,
        ):
            # ---- constants ----
            ident = cp.tile([P, P], F32)
            nc.gpsimd.memset(ident[:, :], 0.0)
            nc.gpsimd.affine_select(out=ident[:, :], in_=ident[:, :],
                                    compare_op=ALU.not_equal, fill=1.0, base=0,
                                    pattern=[[-1, P]], channel_multiplier=1)
            ident_mm = cp.tile([P, P], BF16)
            nc.vector.tensor_copy(out=ident_mm[:, :], in_=ident[:, :])
            ones_row = cp.tile([1, P], F32)
            nc.vector.memset(ones_row[:, :], 1.0)
            winT = cp.tile([P, KCH, H], BF16)
            nc.sync.dma_start(out=winT[:, :, :],
                              in_=winT_e.ap().rearrange("(k p) h -> p k h", p=P))
            b_in_col = cp.tile([H, 1], F32)
            nc.sync.dma_start(out=b_in_col[:, :], in_=binc_e.ap())
            b_out = cp.tile([1, C], F32)
            nc.sync.dma_start(out=b_out[:, :], in_=bout_e.ap())
            woutT = cp.tile([H, C], BF16)
            nc.sync.dma_start(out=woutT[:, :], in_=woutT_e.ap())
            wp = []
            wfT = []
            for l in range(2):
                w1 = cp.tile([H, 3 * H], BF16, tag=f"wp{l}")
                nc.sync.dma_start(out=w1[:, :], in_=wp_e[l].ap())
                wp.append(w1)
                w2 = cp.tile([H, H], BF16, tag=f"wfT{l}")
                nc.sync.dma_start(out=w2[:, :], in_=wfT_e[l].ap())
                wfT.append(w2)

            bo_ps = ps.tile([P, C], F32, tag="mm")
            nc.tensor.matmul(out=bo_ps[:, :], lhsT=ones_row[:, :], rhs=b_out[:, :],
                             start=True, stop=True)
            b_out_full = cp.tile([P, C], F32)
            nc.vector.tensor_copy(out=b_out_full[:, :], in_=bo_ps[:, :])

            # ---- persistent buffers ----
            QT = bp.tile([H, S, P], BF16)
            Qb = bp.tile([P, S, H], F32)
            LG = bp.tile([P, S, C], F32)
            H4 = bp.tile([P, S, H], BF16)

            def body():
                _body(nc, tc, groups, X16, W_buf, R_max, X_max,
                      ident, ident_mm, winT, b_in_col, woutT, wp, wfT,
                      b_out_full, QT, Qb, LG, H4, tabA_own, tabA_full,
                      xT_e, idx16_e, out_e,
                      cp, bp, stp, xp, tbp, gp, pp, ip, dp, sp, ps, psa)

            for _ in range(repeat):
                body()

    nc.compile()
    return nc


def _table_block(nc, l, s, QT, H4, wp, tabA_own, tabA_full, tbp, ps, state):
    """Build table rows for block s; fire AllGather chunks as they complete."""
    tb_ps = ps.tile([P, 3 * H], F32, tag="mm")
    nc.tensor.matmul(out=tb_ps[:, :], lhsT=QT[:, s, :], rhs=wp[l][:, :],
                     start=True, stop=True)
    if s % 2 == 0:
        tba = tbp.tile([P, 2, 2 * H], BF16, tag="tba")
        state["tba"] = tba
        nc.vector.tensor_copy(out=tba[:, 0, :], in_=tb_ps[:, 0:2 * H])
        nc.scalar.activation(out=H4[:, s, :], in_=tb_ps[:, 2 * H:3 * H],
                             func=ACTF.Copy)
    else:
        tba = state["tba"]
        nc.scalar.activation(out=tba[:, 1, :], in_=tb_ps[:, 0:2 * H],
                             func=ACTF.Copy)
        nc.vector.tensor_copy(out=H4[:, s, :], in_=tb_ps[:, 2 * H:3 * H])
        nc.sync.dma_start(
            out=tabA_own[l].ap()[(s - 1) * P:(s + 1) * P, :].rearrange(
                "(b p) h -> p b h", p=P),
            in_=tba[:, :, :])
        if s in AG_FIRE:
            k = AG_FIRE[s]
            nc.gpsimd.collective_compute(
                "AllGather", ALU.bypass,
                replica_groups=[list(range(NCORE))],
                ins=[tabA_own[l].ap()[k * HS:(k + 1) * HS, :]],
                outs=[tabA_full[l].ap()[k * NCORE * HS:(k + 1) * NCORE * HS, :]],
            )


def _body(nc, tc, groups, X16, W_buf, R_max, X_max,
          ident, ident_mm, winT, b_in_col, woutT, wp, wfT,
          b_out_full, QT, Qb, LG, H4, tabA_own, tabA_full,
          xT_e, idx16_e, out_e,
          cp, bp, stp, xp, tbp, gp, pp, ip, dp, sp, ps, psa):
    pad_of_s = {s: (p0, p1) for s, p0, p1 in PAD_ZERO}

    for l in range(2):
        state = {}
        if l == 0:
            # ---- phase 1 fused with table build ----
            for s0 in range(0, S, 2):
                nb2 = min(2, S - s0)
                xt = xp.tile([P, KCH, 2 * P], BF16)
                nc.sync.dma_start(
                    out=xt[:, :, :nb2 * P],
                    in_=xT_e.ap().rearrange("(k p) n -> p k n", p=P)[
                        :, :, s0 * P:(s0 + nb2) * P])
                for s in range(s0, s0 + nb2):
                    off = (s - s0) * P
                    qt_ps = ps.tile([H, P], F32, tag="mm")
                    for kc in range(KCH):
                        nc.tensor.matmul(out=qt_ps[:, :], lhsT=winT[:, kc, :],
                                         rhs=xt[:, kc, off:off + P],
                                         start=(kc == 0), stop=(kc == KCH - 1))
                    nc.scalar.activation(out=QT[:, s, :], in_=qt_ps[:, :],
                                         func=ACTF.Identity, bias=b_in_col[:, :])
                    if s in pad_of_s:
                        p0, p1 = pad_of_s[s]
                        nc.vector.memset(QT[:, s, p0:p1], 0.0)
                    _table_block(nc, l, s, QT, H4, wp, tabA_own, tabA_full,
                                 tbp, ps, state)
        else:
            for s in range(S):
                tr_ps = ps.tile([H, P], F32, tag="mm")
                nc.tensor.transpose(out=tr_ps[:, :], in_=Qb[:, s, :],
                                    identity=ident[:, :])
                nc.scalar.activation(out=QT[:, s, :], in_=tr_ps[:, :],
                                     func=ACTF.Copy)
                _table_block(nc, l, s, QT, H4, wp, tabA_own, tabA_full,
                             tbp, ps, state)

        normsq = stp.tile([P, S], F32, tag="normsq")

        # ---- edge pass over groups ----
        for g in groups:
            s0, nb, tvec, coffs = g["s0"], g["nb"], g["tvec"], g["coffs"]
            Wg = g["W_band"]
            idxt = ip.tile([P, X_max], I16, tag="idx")
            nc.sync.dma_start(
                out=idxt[:, :g["col_cnt"]],
                in_=idx16_e.ap()[:, g["col_off"]:g["col_off"] + g["col_cnt"]])
            gbuf = gp.tile([P, W_buf, 2 * H], BF16, tag="g")
            regions = []
            off = 0
            for k in range(NL):
                tk = tvec[k]
                if tk == 0:
                    regions.append((0, 0))
                    continue
                n = nb * tk * P
                nc.gpsimd.dma_gather(
                    out_ap=gbuf[:, off:off + nb * tk, :],
                    in_ap=tabA_full[l].ap()[
                        k * NCORE * HS:(k + 1) * NCORE * HS, :],
                    idxs_ap=idxt[:, coffs[k]:coffs[k] + n // 16],
                    num_idxs=n, num_idxs_reg=n, elem_size=2 * H,
                    queue_num=0, single_packet=False)
                regions.append((off, nb * tk))
                off += nb * tk

            d = dp.tile([P, W_buf], F32, tag="d")
            for k in range(NL):
                a, n3 = regions[k]
                if n3 == 0:
                    continue
                tk = tvec[k]
                prod = pp.tile([P, R_max, H], BF16, tag="prod")
                g4 = gbuf[:, a:a + n3, 0:H].rearrange(
                    "p (s t) h -> p s t h", t=tk)
                h4b = H4[:, s0:s0 + nb, None, :].to_broadcast([P, nb, tk, H])
                pr4 = prod[:, :n3, :].rearrange("p (s t) h -> p s t h", t=tk)
                nc.vector.tensor_tensor(out=pr4, in0=g4, in1=h4b, op=ALU.mult)
                nc.vector.tensor_reduce(out=d[:, a:a + n3],
                                        in_=prod[:, :n3, :], axis=AX.X,
                                        op=ALU.add)
            sg = dp.tile([P, W_buf], BF16, tag="sg")
            nc.scalar.activation(out=sg[:, :Wg], in_=d[:, :Wg],
                                 func=ACTF.Sigmoid)
            for k in range(NL):
                a, n3 = regions[k]
                if n3 == 0:
                    continue
                nc.vector.tensor_tensor(
                    out=gbuf[:, a:a + n3, H:2 * H],
                    in0=gbuf[:, a:a + n3, H:2 * H],
                    in1=sg[:, a:a + n3].to_broadcast([P, n3, H]), op=ALU.mult)

            # ---- per-block scatter + hloop ----
            band_order = sorted(range(NL), key=lambda k: -tvec[k])
            njmax = min(8, max(tvec))
            for sl in range(nb):
                s = s0 + sl
                agg = psa.tile([P, 8, H], F32, tag="agg")
                chunks = []
                for half in (0, H):
                    for k in band_order:
                        tk = tvec[k]
                        if tk == 0:
                            continue
                        base = regions[k][0] + sl * tk
                        for c0 in range(0, tk, 8):
                            k8 = min(8, tk - c0)
                            chunks.append((base + c0, k8, half))
                for i, (b0, k8, half) in enumerate(chunks):
                    nc.tensor.matmul(
                        out=agg[:, :k8, :], lhsT=ident_mm[:, :],
                        rhs=gbuf[:, b0:b0 + k8, half:half + H],
                        start=(i == 0), stop=(i == len(chunks) - 1))

                hl_ps = ps.tile([P, H], F32, tag="mm")
                nc.tensor.matmul(out=hl_ps[:, :], lhsT=QT[:, s, :],
                                 rhs=wfT[l][:, :], start=True, stop=True)
                hl = sp.tile([P, H], F32, tag="hl")
                nc.scalar.activation(out=hl[:, :], in_=hl_ps[:, :],
                                     func=ACTF.Relu)
                red = sp.tile([P, H], F32, tag="red")
                nc.vector.tensor_reduce(out=red[:, :],
                                        in_=agg[:, :njmax, :].transpose([0, 2, 1]),
                                        axis=AX.X, op=ALU.add)
                nc.vector.tensor_add(out=Qb[:, s, :], in0=hl[:, :], in1=red[:, :])
                qtr = sp.tile([P, H], F32, tag="qtr")
                nc.scalar.activation(out=qtr[:, :], in_=Qb[:, s, :],
                                     func=ACTF.Square,
                                     accum_out=normsq[:, s:s + 1])
                if l == 1:
                    tr_ps = ps.tile([H, P], F32, tag="mm")
                    nc.tensor.transpose(out=tr_ps[:, :], in_=Qb[:, s, :],
                                        identity=ident[:, :])
                    q2t = sp.tile([H, P], BF16, tag="q2t")
                    nc.scalar.activation(out=q2t[:, :], in_=tr_ps[:, :],
                                         func=ACTF.Copy)
                    lg_ps = ps.tile([P, C], F32, tag="mm")
                    nc.tensor.matmul(out=lg_ps[:, :], lhsT=q2t[:, :],
                                     rhs=woutT[:, :], start=True, stop=True)
                    nc.scalar.activation(out=LG[:, s, :], in_=lg_ps[:, :],
                                         func=ACTF.Copy)

        # ---- normalize ----
        ns2 = stp.tile([P, S], F32, tag="ns2")
        nc.vector.tensor_scalar_max(out=ns2[:, :], in0=normsq[:, :], scalar1=1e-24)
        nrm = stp.tile([P, S], F32, tag="nrm")
        nc.scalar.activation(out=nrm[:, :], in_=ns2[:, :], func=ACTF.Sqrt)
        inv = stp.tile([P, S], F32, tag="inv")
        nc.vector.reciprocal(out=inv[:, :], in_=nrm[:, :])
        if l == 0:
            nc.vector.tensor_tensor(out=Qb[:, :, :], in0=Qb[:, :, :],
                                    in1=inv[:, :].to_broadcast([P, S, H]),
                                    op=ALU.mult)
        else:
            nc.vector.tensor_tensor(out=LG[:, :, :], in0=LG[:, :, :],
                                    in1=inv[:, :].to_broadcast([P, S, C]),
                                    op=ALU.mult)
            nc.vector.tensor_tensor(
                out=LG[:, :, :], in0=LG[:, :, :],
                in1=b_out_full[:, None, :].to_broadcast([P, S, C]), op=ALU.add)

    # ---- log_softmax ----
    mx = stp.tile([P, S], F32, tag="mx")
    nc.vector.tensor_reduce(out=mx[:, :], in_=LG[:, :, :], axis=AX.X, op=ALU.max)
    nc.vector.tensor_tensor(out=LG[:, :, :], in0=LG[:, :, :],
                            in1=mx[:, :].to_broadcast([P, S, C]), op=ALU.subtract)
    sume = stp.tile([P, S], F32, tag="sume")
    for s in range(S):
        etr = sp.tile([P, C], F32, tag="etr")
        nc.scalar.activation(out=etr[:, :], in_=LG[:, s, :], func=ACTF.Exp,
                             accum_out=sume[:, s:s + 1])
    lse = stp.tile([P, S], F32, tag="lse")
    nc.scalar.activation(out=lse[:, :], in_=sume[:, :], func=ACTF.Ln)
    nc.vector.tensor_tensor(out=LG[:, :, :], in0=LG[:, :, :],
                            in1=lse[:, :].to_broadcast([P, S, C]), op=ALU.subtract)
    nc.sync.dma_start(out=out_e.ap().rearrange("(s p) c -> p s c", p=P),
                      in_=LG[:, :, :])


# ---------------------------------------------------------------------------
# entry points
# ---------------------------------------------------------------------------

_PLAN_CACHE = {}
_NC_CACHE = {}


def _get_plan(x, edge_index):
    key = hash(np.asarray(edge_index).tobytes())
    if key not in _PLAN_CACHE:
        _PLAN_CACHE[key] = _prep(x, edge_index)
    return _PLAN_CACHE[key]


def _make_inmaps(inputs):
    import ml_dtypes
    x = np.asarray(inputs["x"], np.float32)
    edge_index = np.asarray(inputs["edge_index"])
    plan, idx16_hosts, xT_hosts, node_at = _get_plan(x, edge_index)

    nckey = ("nc", plan["tiles"], str(plan["groups"]))
    if nckey not in _NC_CACHE:
        _NC_CACHE[nckey] = _build(plan)
    nc = _NC_CACHE[nckey]

    W1 = [np.asarray(inputs["W1_0"], np.float32), np.asarray(inputs["W1_1"], np.float32)]
    W2 = [np.asarray(inputs["W2_0"], np.float32), np.asarray(inputs["W2_1"], np.float32)]
    Wf = [np.asarray(inputs["Wf_0"], np.float32), np.asarray(inputs["Wf_1"], np.float32)]
    bf = ml_dtypes.bfloat16
    common = {
        "winT": np.ascontiguousarray(np.asarray(inputs["W_in"], np.float32).T).astype(bf),
        "b_in_col": np.asarray(inputs["b_in"], np.float32).reshape(H, 1),
        "b_out": np.asarray(inputs["b_out"], np.float32).reshape(1, C),
        "woutT": np.ascontiguousarray(np.asarray(inputs["W_out"], np.float32).T).astype(bf),
    }
    for l in range(2):
        # table rows are [h3 | -(h3+h4)]; H4 third block
        common[f"wp{l}"] = np.ascontiguousarray(np.concatenate(
            [W1[l].T, -(W1[l] + W2[l]).T, W2[l].T], axis=1)).astype(bf)
        common[f"wfT{l}"] = np.ascontiguousarray(Wf[l].T).astype(bf)

    in_maps = [dict(common, xT=xT_hosts[c].astype(bf), idx16=idx16_hosts[c])
               for c in range(NCORE)]
    return nc, in_maps, node_at, plan


def kernel(**inputs):
    nc, in_maps, node_at, plan = _make_inmaps(inputs)
    res = run_bass_kernel_spmd(nc, in_maps, core_ids=list(range(NCORE)))

    out = np.empty((N, C), np.float32)
    for c in range(NCORE):
        ids = node_at[c]
        m = ids >= 0
        out[ids[m]] = res.results[c]["out"][m]
    return out
